# revision 20
# baseline (speedup 1.0000x reference)
"""CFConv (gnn message passing) Trainium2 kernel.

Math (per batch b):
    f1 = ssp(r @ W1 + b1)            ssp(x) = softplus(x) - log2
    f2 = ssp(f1 @ W2 + b2)
    out[i, d] = sum_j x[j, d] * f2[i, j, d]

Sharding: data-parallel over batch B=8 across the 8 cores (one batch each).

softplus = Ln(Exp(z + bias) + 1) on the ACT engine (Exp and Ln share the
natural_log_exp table set; the chooser is pinned to it).  Measured ACT
throughput is ~0.575us per 1024-col op (~2 elem/lane/cycle), so 4 ACT passes
(~150us/core) beat every sigmoid/custom-DVE variant whose fused finish op
runs at 1 elem/cycle on the slower DVE.  Layer biases ride the Exp affine
(bias slot) so the PE does only data matmuls.  The "-log2" shifts fold
host-side: layer 1's into b2' = b2 - log2*sum_d W2[d,:], layer 2's into
corr = -log2*sum_j x[j,d] added at the output.

Per-core pipeline (features on partitions, (i,j)-pairs on the free dim):
  r pairs are DMA-transposed to SBUF [128, pairs] bf16 (even j in partitions
  0:64, odd j in 64:128), in 4096-pair chunks.  Work flows in 1024-col
  half-groups (one j-parity of 8 query rows): mm1 (K=64) -> z1 in a 2-bank
  PSUM tile -> Exp -> bf16 e1 -> Ln -> a1; mm2 (K=128) -> z2 -> Exp -> Ln ->
  f2 -> f2*xT (DVE, 2 elem/cyc).  Both PSUM pools are double-buffered
  (2+2 tiles x 2 banks = 8 banks) and PSUM tiles are freed by the Exp, so
  the in-order engine queues pipeline with two steps of slack everywhere.
  Layer-2 work of chunk c-1 interleaves with layer-1 of chunk c.

The j-reduction of prod runs as an f32 add-tree off the DVE/ACT critical
path: level 1 (128->64) as one DVE op per chunk, levels 2-7 plus the
even/odd combine and the corr add on the otherwise-idle GPSIMD (Pool)
engine.  Output stays [d, i] on device; the host transposes back.
"""

import numpy as np
import ml_dtypes

import concourse.bass as bass
import concourse.tile as tile
from concourse import bacc, mybir
from concourse.bass_utils import run_bass_kernel_spmd

LOG2 = float(np.log(2.0))

B, N, D, RBF = 8, 256, 128, 64
PAIRS = N * N // 2            # 32768 row-pairs per batch
CHUNK_PAIRS = 4096            # pairs per DMA-transpose chunk (1 MiB)
GROUP_PAIRS = 1024            # pairs per half-group (8 query nodes i)
SUB = 512                     # cols per matmul (one PSUM bank)
HG = 1024                     # cols per PSUM half-group tile
I_PER_GROUP = GROUP_PAIRS // (N // 2)   # 8
H = CHUNK_PAIRS // GROUP_PAIRS          # groups per chunk tile (4)
N_CORES = 8

BF16 = mybir.dt.bfloat16
F32 = mybir.dt.float32


def _build_program(reps: int = 1, unroll: int = 1):
    # Restrict the ACT-table chooser to the one set holding BOTH Exp and Ln;
    # otherwise it can alternate between per-function sets and pay a ~2.7us
    # table load on every activation.
    import concourse.bacc as _bacc_mod
    from concourse.hw_specs import get_activation_tables as _gat
    _orig = _gat("gen3")
    _both = mybir.ActivationFunctionType.Exp, mybir.ActivationFunctionType.Ln
    _patched = {
        name: (funcs if name == "natural_log_exp_and_others"
               else type(funcs)(f for f in funcs if f not in _both))
        for name, funcs in _orig.items()
    }
    _bacc_mod.get_activation_tables = lambda arch: _patched

    nc = bacc.Bacc("TRN2", target_bir_lowering=False, debug=False,
                   num_devices=N_CORES)

    rp = nc.dram_tensor("rp", [PAIRS, 2 * RBF], BF16, kind="ExternalInput").ap()
    xte = nc.dram_tensor("xte", [D, N // 2], BF16, kind="ExternalInput").ap()
    xto = nc.dram_tensor("xto", [D, N // 2], BF16, kind="ExternalInput").ap()
    corr = nc.dram_tensor("corr", [D, 1], F32, kind="ExternalInput").ap()
    w1s = nc.dram_tensor("w1s", [2 * RBF, D], BF16, kind="ExternalInput").ap()
    w2 = nc.dram_tensor("w2", [D, D], BF16, kind="ExternalInput").ap()
    b1c = nc.dram_tensor("b1c", [D, 1], F32, kind="ExternalInput").ap()
    b2p = nc.dram_tensor("b2p", [D, 1], F32, kind="ExternalInput").ap()
    outT = nc.dram_tensor("outT", [D, N], F32, kind="ExternalOutput").ap()

    f_exp = mybir.ActivationFunctionType.Exp
    f_ln = mybir.ActivationFunctionType.Ln
    mult = mybir.AluOpType.mult
    add = mybir.AluOpType.add

    with tile.TileContext(nc) as tc:
        with (
            tc.tile_pool(name="const", bufs=1) as const,
            tc.tile_pool(name="rt", bufs=3) as rt_pool,
            tc.tile_pool(name="e1", bufs=2) as e1_pool,
            tc.tile_pool(name="e2", bufs=2) as e2_pool,
            tc.tile_pool(name="a1", bufs=2) as a1_pool,
            tc.tile_pool(name="f2", bufs=2) as f2_pool,
            tc.tile_pool(name="prod", bufs=2) as prod_pool,
            tc.tile_pool(name="t0p", bufs=2) as t0_pool,
            tc.tile_pool(name="tree", bufs=1) as tree_pool,
            tc.tile_pool(name="acc", bufs=2) as acc_pool,
            tc.tile_pool(name="osb", bufs=1) as out_pool,
            tc.tile_pool(name="z1", bufs=2, space="PSUM") as z1_pool,
            tc.tile_pool(name="z2", bufs=2, space="PSUM") as z2_pool,
        ):
            w1s_t = const.tile([2 * RBF, D], BF16, tag="w1s")
            w2_t = const.tile([D, D], BF16, tag="w2")
            xte_t = const.tile([D, N // 2], BF16, tag="xte")
            xto_t = const.tile([D, N // 2], BF16, tag="xto")
            b1_t = const.tile([D, 1], F32, tag="b1")
            b2p_t = const.tile([D, 1], F32, tag="b2p")
            corr_t = const.tile([D, 1], F32, tag="corr")
            nc.sync.dma_start(w1s_t[:], w1s[:])
            nc.sync.dma_start(b1_t[:], b1c[:])
            nc.sync.dma_start(w2_t[:], w2[:])
            nc.sync.dma_start(xte_t[:], xte[:])
            nc.sync.dma_start(xto_t[:], xto[:])
            nc.sync.dma_start(b2p_t[:], b2p[:])
            nc.sync.dma_start(corr_t[:], corr[:])

            out_sb = out_pool.tile([D, N], F32, tag="osb")

            # Tiny warmup activation right after the const loads: hoists the
            # ~2.7us ACT table load to t~0 where it overlaps the first DMA.
            warm = acc_pool.tile([D, 1], F32, tag="warm")
            nc.scalar.activation(warm[:], b1_t[:], f_exp, bias=0.0)

            jw = N // 2
            PW = H * 2 * HG               # z-cols per chunk tile (8192)
            I_PAIR = H * I_PER_GROUP      # 32 query nodes per chunk tile
            M = PW // jw                  # reduce segments per chunk (64)

            def stage1_half(rt, a1w, hh):
                """mm1 -> Exp(+b1) -> Ln for one 1024-col half-group (one
                j-parity of one group) of the current chunk."""
                h, par = hh // 2, hh % 2
                g0 = h * GROUP_PAIRS
                r0, r1 = par * RBF, (par + 1) * RBF
                z1 = z1_pool.tile([D, HG], F32, tag="z1")
                for s in range(HG // SUB):
                    cs = g0 + s * SUB
                    nc.tensor.matmul(
                        z1[:, s * SUB:(s + 1) * SUB],
                        w1s_t[r0:r1, :],
                        rt[r0:r1, cs:cs + SUB],
                    )
                e1 = e1_pool.tile([D, HG], BF16, tag="e1")
                nc.scalar.activation(e1[:], z1[:], f_exp, bias=b1_t[:])
                nc.scalar.activation(
                    a1w[:, hh * HG:(hh + 1) * HG], e1[:], f_ln, bias=1.0)

            def stage2_half(a1w, f2w, prod, hh):
                """mm2 -> Exp(+b2') -> Ln -> *x for one 1024-col half-group
                of the previous chunk."""
                c0 = hh * HG
                z2 = z2_pool.tile([D, HG], F32, tag="z2")
                for s in range(HG // SUB):
                    nc.tensor.matmul(
                        z2[:, s * SUB:(s + 1) * SUB],
                        w2_t[:],
                        a1w[:, c0 + s * SUB:c0 + (s + 1) * SUB],
                    )
                e2 = e2_pool.tile([D, HG], BF16, tag="e2")
                nc.scalar.activation(e2[:], z2[:], f_exp, bias=b2p_t[:])
                nc.scalar.activation(
                    f2w[:, c0:c0 + HG], e2[:], f_ln, bias=1.0)
                xb = (xte_t if hh % 2 == 0 else xto_t)[:, None, :]
                nc.vector.tensor_tensor(
                    prod[:, c0:c0 + HG].rearrange("p (k j) -> p k j", j=jw),
                    f2w[:, c0:c0 + HG].rearrange("p (k j) -> p k j", j=jw),
                    xb.broadcast_to([D, I_PER_GROUP, jw]),
                    mult,
                )

            def chunk_tail(prod, i0):
                """j-reduction of one chunk's f2*x products: level 1 of the
                f32 add-tree on DVE (one op), levels 2-7 plus the even/odd
                combine and corr add on the otherwise-idle Pool engine."""
                m3 = prod[:].rearrange("p (m j) -> p m j", j=jw)
                t0 = t0_pool.tile([D, M, jw // 2], F32, tag="t0")
                nc.vector.tensor_tensor(
                    t0[:], m3[:, :, 0:jw // 2], m3[:, :, jw // 2:jw], add)
                t = t0[:]
                for lvl in range(1, 7):
                    half = jw >> (lvl + 1)
                    nxt = tree_pool.tile([D, M, half], F32, tag=f"t{lvl}")
                    nc.gpsimd.tensor_add(
                        nxt[:], t[:, :, 0:half], t[:, :, half:2 * half])
                    t = nxt
                # t is [D, M, 1]; segments m = h*16 + par*8 + k
                s4 = t[:].rearrange(
                    "p (h par k) o -> p h par (k o)", h=H, par=2)
                tmp = acc_pool.tile([D, I_PAIR], F32, tag="tmp")
                nc.gpsimd.tensor_add(
                    tmp[:].rearrange("p (h k) -> p h k", h=H),
                    s4[:, :, 0, :], s4[:, :, 1, :])
                nc.gpsimd.tensor_scalar_add(
                    out_sb[:, i0:i0 + I_PAIR], tmp[:], corr_t[:])

            def group_tail(prod, g, i0):
                """per-group variant of chunk_tail used in the final flush so
                the tree overlaps the remaining layer-2 work."""
                m3 = prod[:].rearrange("p (m j) -> p m j", j=jw)
                t = m3[:, 16 * g:16 * (g + 1), :]
                for lvl in range(7):
                    half = jw >> (lvl + 1)
                    nxt = tree_pool.tile([D, 16, half], F32, tag=f"g{lvl}")
                    nc.gpsimd.tensor_add(
                        nxt[:], t[:, :, 0:half], t[:, :, half:2 * half])
                    t = nxt
                s4 = t[:].rearrange("p (par k) o -> p par (k o)", par=2)
                tmp = acc_pool.tile([D, I_PER_GROUP], F32, tag="tmpg")
                nc.gpsimd.tensor_add(tmp[:], s4[:, 0, :], s4[:, 1, :])
                nc.gpsimd.tensor_scalar_add(
                    out_sb[:, i0 + 8 * g:i0 + 8 * (g + 1)], tmp[:], corr_t[:])

            # Software-pipelined emission interleaving half-groups of chunk
            # c's layer 1 with half-groups of chunk c-1's layer 2.
            def body():
                pending = None  # (a1w, i0) of the previous chunk
                for c in range(PAIRS // CHUNK_PAIRS):
                    rt = rt_pool.tile([2 * RBF, CHUNK_PAIRS], BF16, tag="rt")
                    if c == 0:
                        # Slice the first transpose 8 ways so mm1 of the
                        # first half-group starts as soon as 128 KiB lands.
                        qq = CHUNK_PAIRS // 8
                        for k in range(8):
                            nc.sync.dma_start_transpose(
                                out=rt[:, k * qq:(k + 1) * qq],
                                in_=rp[k * qq:(k + 1) * qq, :],
                            )
                    else:
                        nc.sync.dma_start_transpose(
                            out=rt[:],
                            in_=rp[c * CHUNK_PAIRS:(c + 1) * CHUNK_PAIRS, :],
                        )
                    a1w = a1_pool.tile([D, PW], BF16, tag="a1")
                    if pending is not None:
                        f2w = f2_pool.tile([D, PW], BF16, tag="f2")
                        prod = prod_pool.tile([D, PW], BF16, tag="prod")
                    for h in range(H):
                        stage1_half(rt, a1w, 2 * h)
                        if pending is not None:
                            stage2_half(pending[0], f2w, prod, 2 * h)
                        stage1_half(rt, a1w, 2 * h + 1)
                        if pending is not None:
                            stage2_half(pending[0], f2w, prod, 2 * h + 1)
                    if pending is not None:
                        chunk_tail(prod, pending[1])
                    pending = (a1w, c * I_PAIR)
                # flush the last chunk's layer 2 (z2 pool double-buffers);
                # per-group tails so the Pool tree overlaps the layer-2 work.
                f2w = f2_pool.tile([D, PW], BF16, tag="f2")
                prod = prod_pool.tile([D, PW], BF16, tag="prod")
                for hh in range(2 * H):
                    stage2_half(pending[0], f2w, prod, hh)
                    if hh % 2 == 1:
                        group_tail(prod, hh // 2, pending[1])

            if unroll > 1:
                for _ in range(unroll):
                    body()
            elif reps == 1:
                body()
            else:
                with tc.For_i(0, reps, 1):
                    body()

            nc.sync.dma_start(outT[:], out_sb[:])

    nc.compile()
    return nc


def _prepare_inputs(x, r, W1, b1, W2, b2):
    bf16 = ml_dtypes.bfloat16
    W1 = np.asarray(W1, np.float32)
    W2 = np.asarray(W2, np.float32)
    w1s = np.concatenate([W1, W1], axis=0).astype(bf16)          # [128, 128]
    w2b = W2.astype(bf16)                                        # [128, 128]
    b1c = np.asarray(b1, np.float32).reshape(D, 1)
    b2pv = (np.asarray(b2, np.float64)
            - LOG2 * W2.astype(np.float64).sum(axis=0)
            ).astype(np.float32).reshape(D, 1)

    in_maps = []
    for b in range(B):
        xbT = np.asarray(x[b], np.float32).T                     # [128 d, 256 j]
        in_maps.append({
            "rp": np.ascontiguousarray(
                np.asarray(r[b], np.float32).reshape(PAIRS, 2 * RBF)
            ).astype(bf16),
            "xte": np.ascontiguousarray(xbT[:, 0::2]).astype(bf16),
            "xto": np.ascontiguousarray(xbT[:, 1::2]).astype(bf16),
            "corr": (-LOG2 * xbT.sum(axis=1, dtype=np.float64)
                     ).astype(np.float32).reshape(D, 1),
            "w1s": w1s,
            "w2": w2b,
            "b1c": b1c,
            "b2p": b2pv,
        })
    return in_maps


_NC_CACHE = None


def _get_nc():
    global _NC_CACHE
    if _NC_CACHE is None:
        _NC_CACHE = _build_program()
    return _NC_CACHE


def hw_time_ns(inputs, reps=2049, n_meas=4):
    """Measure on-device per-iteration time by comparing wall time of a
    reps-times device loop against a single-iteration run."""
    import time as _time
    in_maps = _prepare_inputs(**inputs)

    def run_with(nc_prog):
        ts = []
        for _ in range(n_meas):
            t0 = _time.time()
            run_bass_kernel_spmd(nc_prog, in_maps, list(range(N_CORES)))
            ts.append(_time.time() - t0)
        return min(ts)

    nc1 = _build_program(reps=1)
    ncr = _build_program(reps=reps)
    w1 = run_with(nc1)
    wr = run_with(ncr)
    return (wr - w1) / (reps - 1) * 1e9


def kernel(x, r, W1, b1, W2, b2, _trace=False, _trace_kwargs=None):
    nc = _get_nc()
    in_maps = _prepare_inputs(x, r, W1, b1, W2, b2)
    res = run_bass_kernel_spmd(
        nc, in_maps, list(range(N_CORES)),
        trace=_trace, **(_trace_kwargs or {}),
    )
    out = np.stack([
        np.asarray(res.results[b]["outT"], np.float32).T for b in range(B)
    ])
    if _trace:
        return out, res
    return out


# revision 24
# speedup vs baseline: 1.1000x; 1.1000x over previous
"""CFConv (gnn message passing) Trainium2 kernel.

Math (per batch b):
    f1 = ssp(r @ W1 + b1)            ssp(x) = softplus(x) - log2
    f2 = ssp(f1 @ W2 + b2)
    out[i, d] = sum_j x[j, d] * f2[i, j, d]

Sharding: data-parallel over batch B=8 across the 8 cores (one batch each).

softplus = Ln(Exp(z + bias) + 1) on the ACT engine (Exp and Ln share the
natural_log_exp table set; the chooser is pinned to it).  Measured ACT
throughput is ~0.575us per 1024-col op (~2 elem/lane/cycle), so 4 ACT passes
(~150us/core) beat every sigmoid/custom-DVE variant whose fused finish op
runs at 1 elem/cycle on the slower DVE.  Layer biases ride the Exp affine
(bias slot) so the PE does only data matmuls.  The "-log2" shifts fold
host-side: layer 1's into b2' = b2 - log2*sum_d W2[d,:], layer 2's into
corr = -log2*sum_j x[j,d] added at the output.

Per-core pipeline (features on partitions, (i,j)-pairs on the free dim):
  r pairs are DMA-transposed to SBUF [128, pairs] bf16 (even j in partitions
  0:64, odd j in 64:128), in 4096-pair chunks.  Work flows in 1024-col
  half-groups (one j-parity of 8 query rows): mm1 (K=64) -> z1 in a 2-bank
  PSUM tile -> Exp -> bf16 e1 -> Ln -> a1; mm2 (K=128) -> z2 -> Exp -> Ln ->
  f2 -> f2*xT (DVE, 2 elem/cyc).  Both PSUM pools are double-buffered
  (2+2 tiles x 2 banks = 8 banks) and PSUM tiles are freed by the Exp, so
  the in-order engine queues pipeline with two steps of slack everywhere.
  Layer-2 work of chunk c-1 interleaves with layer-1 of chunk c.

The j-reduction of prod runs as an f32 add-tree off the DVE/ACT critical
path: level 1 (128->64) as one DVE op per chunk, levels 2-7 plus the
even/odd combine and the corr add on the otherwise-idle GPSIMD (Pool)
engine.  Output stays [d, i] on device; the host transposes back.
"""

import numpy as np
import ml_dtypes

import concourse.bass as bass
import concourse.tile as tile
from concourse import bacc, mybir
from concourse.bass_utils import run_bass_kernel_spmd

LOG2 = float(np.log(2.0))

B, N, D, RBF = 8, 256, 128, 64
PAIRS = N * N // 2            # 32768 row-pairs per batch
CHUNK_PAIRS = 4096            # pairs per DMA-transpose chunk (1 MiB)
GROUP_PAIRS = 1024            # pairs per half-group (8 query nodes i)
SUB = 512                     # cols per matmul (one PSUM bank)
HG = 1024                     # cols per PSUM half-group tile
I_PER_GROUP = GROUP_PAIRS // (N // 2)   # 8
H = CHUNK_PAIRS // GROUP_PAIRS          # groups per chunk tile (4)
N_CORES = 8

BF16 = mybir.dt.bfloat16
F32 = mybir.dt.float32


def _build_program(reps: int = 1, unroll: int = 1):
    # Restrict the ACT-table chooser to the one set holding BOTH Exp and Ln;
    # otherwise it can alternate between per-function sets and pay a ~2.7us
    # table load on every activation.
    import concourse.bacc as _bacc_mod
    from concourse.hw_specs import get_activation_tables as _gat
    _orig = _gat("gen3")
    _both = mybir.ActivationFunctionType.Exp, mybir.ActivationFunctionType.Ln
    _patched = {
        name: (funcs if name == "natural_log_exp_and_others"
               else type(funcs)(f for f in funcs if f not in _both))
        for name, funcs in _orig.items()
    }
    _bacc_mod.get_activation_tables = lambda arch: _patched

    nc = bacc.Bacc("TRN2", target_bir_lowering=False, debug=False,
                   num_devices=N_CORES)

    rp = nc.dram_tensor("rp", [PAIRS, 2 * RBF], BF16, kind="ExternalInput").ap()
    xte = nc.dram_tensor("xte", [D, N // 2], BF16, kind="ExternalInput").ap()
    xto = nc.dram_tensor("xto", [D, N // 2], BF16, kind="ExternalInput").ap()
    corr = nc.dram_tensor("corr", [D, 1], F32, kind="ExternalInput").ap()
    w1s = nc.dram_tensor("w1s", [2 * RBF, D], BF16, kind="ExternalInput").ap()
    w2 = nc.dram_tensor("w2", [D, D], BF16, kind="ExternalInput").ap()
    b1c = nc.dram_tensor("b1c", [D, 1], F32, kind="ExternalInput").ap()
    b2p = nc.dram_tensor("b2p", [D, 1], F32, kind="ExternalInput").ap()
    outT = nc.dram_tensor("outT", [D, N], F32, kind="ExternalOutput").ap()

    f_exp = mybir.ActivationFunctionType.Exp
    f_ln = mybir.ActivationFunctionType.Ln
    mult = mybir.AluOpType.mult
    add = mybir.AluOpType.add

    with tile.TileContext(nc) as tc:
        with (
            tc.tile_pool(name="const", bufs=1) as const,
            tc.tile_pool(name="rt", bufs=3) as rt_pool,
            tc.tile_pool(name="e1", bufs=1) as e1_pool,
            tc.tile_pool(name="e2", bufs=1) as e2_pool,
            tc.tile_pool(name="a1", bufs=2) as a1_pool,
            tc.tile_pool(name="f2", bufs=2) as f2_pool,
            tc.tile_pool(name="prod", bufs=2) as prod_pool,
            tc.tile_pool(name="t0p", bufs=2) as t0_pool,
            tc.tile_pool(name="tree", bufs=1) as tree_pool,
            tc.tile_pool(name="acc", bufs=2) as acc_pool,
            tc.tile_pool(name="osb", bufs=1) as out_pool,
            tc.tile_pool(name="z1", bufs=2, space="PSUM") as z1_pool,
            tc.tile_pool(name="z2", bufs=2, space="PSUM") as z2_pool,
        ):
            w1s_t = const.tile([2 * RBF, D], BF16, tag="w1s")
            w2_t = const.tile([D, D], BF16, tag="w2")
            xte_t = const.tile([D, N // 2], BF16, tag="xte")
            xto_t = const.tile([D, N // 2], BF16, tag="xto")
            b1_t = const.tile([D, 1], F32, tag="b1")
            b2p_t = const.tile([D, 1], F32, tag="b2p")
            corr_t = const.tile([D, 1], F32, tag="corr")
            nc.sync.dma_start(w1s_t[:], w1s[:])
            nc.sync.dma_start(b1_t[:], b1c[:])
            nc.sync.dma_start(w2_t[:], w2[:])
            nc.sync.dma_start(xte_t[:], xte[:])
            nc.sync.dma_start(xto_t[:], xto[:])
            nc.sync.dma_start(b2p_t[:], b2p[:])
            nc.sync.dma_start(corr_t[:], corr[:])

            out_sb = out_pool.tile([D, N], F32, tag="osb")

            # Tiny warmup activation right after the const loads: hoists the
            # ~2.7us ACT table load to t~0 where it overlaps the first DMA.
            warm = acc_pool.tile([D, 1], F32, tag="warm")
            nc.scalar.activation(warm[:], b1_t[:], f_exp, bias=0.0)

            jw = N // 2
            PW = H * 2 * HG               # z-cols per chunk tile (8192)
            I_PAIR = H * I_PER_GROUP      # 32 query nodes per chunk tile
            M = PW // jw                  # reduce segments per chunk (64)

            def stage1_half(rt, e1w, hh):
                """mm1 -> Exp(+b1) for one 1024-col half-group (one j-parity
                of one group) of the current chunk.  The Ln runs once per
                chunk at FD 8192 to amortize the per-op ACT cost."""
                h, par = hh // 2, hh % 2
                g0 = h * GROUP_PAIRS
                r0, r1 = par * RBF, (par + 1) * RBF
                z1 = z1_pool.tile([D, HG], F32, tag="z1")
                for s in range(HG // SUB):
                    cs = g0 + s * SUB
                    nc.tensor.matmul(
                        z1[:, s * SUB:(s + 1) * SUB],
                        w1s_t[r0:r1, :],
                        rt[r0:r1, cs:cs + SUB],
                    )
                nc.scalar.activation(
                    e1w[:, hh * HG:(hh + 1) * HG], z1[:], f_exp,
                    bias=b1_t[:])

            def stage2_half(a1w, e2w, hh):
                """mm2 -> Exp(+b2') for one 1024-col half-group of the
                previous chunk."""
                c0 = hh * HG
                z2 = z2_pool.tile([D, HG], F32, tag="z2")
                for s in range(HG // SUB):
                    nc.tensor.matmul(
                        z2[:, s * SUB:(s + 1) * SUB],
                        w2_t[:],
                        a1w[:, c0 + s * SUB:c0 + (s + 1) * SUB],
                    )
                nc.scalar.activation(
                    e2w[:, c0:c0 + HG], z2[:], f_exp, bias=b2p_t[:])

            def chunk_prod(f2w, prod):
                """f2 * xT for a whole chunk: two 4-D broadcast TTs on DVE
                (bf16, 2 elem/cyc)."""
                p4 = prod[:].rearrange(
                    "p (h par k j) -> p h par k j", h=H, par=2, j=jw)
                f4 = f2w[:].rearrange(
                    "p (h par k j) -> p h par k j", h=H, par=2, j=jw)
                xe4 = xte_t[:, None, None, :].broadcast_to(
                    [D, H, I_PER_GROUP, jw])
                xo4 = xto_t[:, None, None, :].broadcast_to(
                    [D, H, I_PER_GROUP, jw])
                nc.vector.tensor_tensor(
                    p4[:, :, 0, :, :], f4[:, :, 0, :, :], xe4, mult)
                nc.vector.tensor_tensor(
                    p4[:, :, 1, :, :], f4[:, :, 1, :, :], xo4, mult)

            def chunk_tail(f2w, prod, i0):
                """f2*x then the j-reduction for one chunk: the products and
                level 1 of the f32 add-tree on DVE, levels 2-7 plus the
                even/odd combine and corr add on the otherwise-idle Pool
                engine."""
                chunk_prod(f2w, prod)
                m3 = prod[:].rearrange("p (m j) -> p m j", j=jw)
                t0 = t0_pool.tile([D, M, jw // 2], F32, tag="t0")
                nc.vector.tensor_tensor(
                    t0[:], m3[:, :, 0:jw // 2], m3[:, :, jw // 2:jw], add)
                t = t0[:]
                for lvl in range(1, 7):
                    half = jw >> (lvl + 1)
                    nxt = tree_pool.tile([D, M, half], F32, tag=f"t{lvl}")
                    nc.gpsimd.tensor_add(
                        nxt[:], t[:, :, 0:half], t[:, :, half:2 * half])
                    t = nxt
                # t is [D, M, 1]; segments m = h*16 + par*8 + k
                s4 = t[:].rearrange(
                    "p (h par k) o -> p h par (k o)", h=H, par=2)
                tmp = acc_pool.tile([D, I_PAIR], F32, tag="tmp")
                nc.gpsimd.tensor_add(
                    tmp[:].rearrange("p (h k) -> p h k", h=H),
                    s4[:, :, 0, :], s4[:, :, 1, :])
                nc.gpsimd.tensor_scalar_add(
                    out_sb[:, i0:i0 + I_PAIR], tmp[:], corr_t[:])

            # Software-pipelined emission interleaving half-groups of chunk
            # c's layer 1 with half-groups of chunk c-1's layer 2.
            def body():
                pending = None  # (a1w, i0) of the previous chunk
                for c in range(PAIRS // CHUNK_PAIRS):
                    rt = rt_pool.tile([2 * RBF, CHUNK_PAIRS], BF16, tag="rt")
                    if c == 0:
                        # Slice the first transpose 8 ways so mm1 of the
                        # first half-group starts as soon as 128 KiB lands.
                        qq = CHUNK_PAIRS // 8
                        for k in range(8):
                            nc.sync.dma_start_transpose(
                                out=rt[:, k * qq:(k + 1) * qq],
                                in_=rp[k * qq:(k + 1) * qq, :],
                            )
                    else:
                        nc.sync.dma_start_transpose(
                            out=rt[:],
                            in_=rp[c * CHUNK_PAIRS:(c + 1) * CHUNK_PAIRS, :],
                        )
                    e1w = e1_pool.tile([D, PW], BF16, tag="e1")
                    if pending is not None:
                        e2w = e2_pool.tile([D, PW], BF16, tag="e2")
                    for h in range(H):
                        stage1_half(rt, e1w, 2 * h)
                        if pending is not None:
                            stage2_half(pending[0], e2w, 2 * h)
                        stage1_half(rt, e1w, 2 * h + 1)
                        if pending is not None:
                            stage2_half(pending[0], e2w, 2 * h + 1)
                    if pending is not None:
                        f2w = f2_pool.tile([D, PW], BF16, tag="f2")
                        nc.scalar.activation(f2w[:], e2w[:], f_ln, bias=1.0)
                    a1w = a1_pool.tile([D, PW], BF16, tag="a1")
                    nc.scalar.activation(a1w[:], e1w[:], f_ln, bias=1.0)
                    if pending is not None:
                        prod = prod_pool.tile([D, PW], BF16, tag="prod")
                        chunk_tail(f2w, prod, pending[1])
                    pending = (a1w, c * I_PAIR)
                # flush the last chunk's layer 2 (z2 pool double-buffers).
                e2w = e2_pool.tile([D, PW], BF16, tag="e2")
                for hh in range(2 * H):
                    stage2_half(pending[0], e2w, hh)
                f2w = f2_pool.tile([D, PW], BF16, tag="f2")
                nc.scalar.activation(f2w[:], e2w[:], f_ln, bias=1.0)
                prod = prod_pool.tile([D, PW], BF16, tag="prod")
                chunk_tail(f2w, prod, pending[1])

            if unroll > 1:
                for _ in range(unroll):
                    body()
            elif reps == 1:
                body()
            else:
                with tc.For_i(0, reps, 1):
                    body()

            nc.sync.dma_start(outT[:], out_sb[:])

    nc.compile()
    return nc


def _prepare_inputs(x, r, W1, b1, W2, b2):
    bf16 = ml_dtypes.bfloat16
    W1 = np.asarray(W1, np.float32)
    W2 = np.asarray(W2, np.float32)
    w1s = np.concatenate([W1, W1], axis=0).astype(bf16)          # [128, 128]
    w2b = W2.astype(bf16)                                        # [128, 128]
    b1c = np.asarray(b1, np.float32).reshape(D, 1)
    b2pv = (np.asarray(b2, np.float64)
            - LOG2 * W2.astype(np.float64).sum(axis=0)
            ).astype(np.float32).reshape(D, 1)

    in_maps = []
    for b in range(B):
        xbT = np.asarray(x[b], np.float32).T                     # [128 d, 256 j]
        in_maps.append({
            "rp": np.ascontiguousarray(
                np.asarray(r[b], np.float32).reshape(PAIRS, 2 * RBF)
            ).astype(bf16),
            "xte": np.ascontiguousarray(xbT[:, 0::2]).astype(bf16),
            "xto": np.ascontiguousarray(xbT[:, 1::2]).astype(bf16),
            "corr": (-LOG2 * xbT.sum(axis=1, dtype=np.float64)
                     ).astype(np.float32).reshape(D, 1),
            "w1s": w1s,
            "w2": w2b,
            "b1c": b1c,
            "b2p": b2pv,
        })
    return in_maps


_NC_CACHE = None


def _get_nc():
    global _NC_CACHE
    if _NC_CACHE is None:
        _NC_CACHE = _build_program()
    return _NC_CACHE


def hw_time_ns(inputs, reps=2049, n_meas=4):
    """Measure on-device per-iteration time by comparing wall time of a
    reps-times device loop against a single-iteration run."""
    import time as _time
    in_maps = _prepare_inputs(**inputs)

    def run_with(nc_prog):
        ts = []
        for _ in range(n_meas):
            t0 = _time.time()
            run_bass_kernel_spmd(nc_prog, in_maps, list(range(N_CORES)))
            ts.append(_time.time() - t0)
        return min(ts)

    nc1 = _build_program(reps=1)
    ncr = _build_program(reps=reps)
    w1 = run_with(nc1)
    wr = run_with(ncr)
    return (wr - w1) / (reps - 1) * 1e9


def kernel(x, r, W1, b1, W2, b2, _trace=False, _trace_kwargs=None):
    nc = _get_nc()
    in_maps = _prepare_inputs(x, r, W1, b1, W2, b2)
    res = run_bass_kernel_spmd(
        nc, in_maps, list(range(N_CORES)),
        trace=_trace, **(_trace_kwargs or {}),
    )
    out = np.stack([
        np.asarray(res.results[b]["outT"], np.float32).T for b in range(B)
    ])
    if _trace:
        return out, res
    return out
